# revision 38
# baseline (speedup 1.0000x reference)
"""Trainium2 Bass kernel for additive (Bahdanau-style) attention aggregation.

Reference per batch b:
    qe = query @ Wq + bq; me = memory @ Wm + bm
    S[q,m] = sum_d wst[d] * tanh(qe[q,d] + me[m,d])
    out = softmax(S, m) @ memory

Sharding: data-parallel over batch B=8, one element per NeuronCore.

Algorithm: tanh(x) ~= C1 sin(Wx) + C3 sin(3Wx) fitted with a Gaussian-
density weight on the data's x-range (|x|<=4.7); each sin(kW(a+b))
separates into sin/cos products, so the score matrix is 4 rank-512
matmul terms on the PE. sin3/cos3 come from a short Chebyshev ladder
(sin3 = (3-4sin^2)sin, cos3 = (1-4sin^2)cos) with products on DVE and
scalar-linear steps on GpSimd. Scores are computed TRANSPOSED ([m,q] in
two PSUM half-tiles) so exp(S^T) feeds the output matmul directly as
lhsT -- no PE transposes; the softmax row-sum falls out of an extra
ones-column matmul. The output leaves PSUM unnormalized together with
the row-sums; the host divides. Weights are laid out d_out-major and
k-split so each PSUM bank's encoder inputs arrive as early as possible,
and every consumer is emitted immediately after its producer group
(the Tile scheduler honors emission order per engine and tracks
dependencies against writers-emitted-so-far).
"""

import os
import numpy as np
import ml_dtypes

import concourse.bass as bass
import concourse.bacc as bacc
import concourse.tile as tile
from concourse import mybir
from concourse.bass_utils import run_bass_kernel_spmd

F32 = mybir.dt.float32
BF16 = mybir.dt.bfloat16
F8 = mybir.dt.float8e4
AF = mybir.ActivationFunctionType
OP = mybir.AluOpType

B = 8
LQ = 128
LM = 256
D = 512
KC = D // 128   # d-model chunks
MH = LM // 128  # memory partition chunks
PIH = float(np.pi / 2)

# tanh(x) ~= C1 sin(Wx) + C3 sin(3Wx), density-weighted fit on |x|<=4.7
W = 0.686790
C1, C3 = 1.056331, 0.115109
if os.environ.get("KERNEL_SIM_SAFE"):  # CoreSim asserts |sin arg| <= pi;
    W = 0.54926                        # HW degrades gracefully past pi
    C1, C3 = 1.114898, 0.19142
R31 = C3 / C1
MASK_NEG = 50.0


def _build() -> bass.Bass:
    nc = bacc.Bacc("TRN2", target_bir_lowering=False)

    # wm: d_out-major halves (separate completions); qbig: qT|wqL|wqR
    mT_d = nc.declare_dram_parameter("mT", [128, KC * LM], BF16, isOutput=False)
    wmL_d = nc.declare_dram_parameter("wmL", [128, KC * LM], F8, isOutput=False)
    wmR_d = nc.declare_dram_parameter("wmR", [128, KC * LM], F8, isOutput=False)
    qT_d = nc.declare_dram_parameter("qT", [128, D], BF16, isOutput=False)
    wq8_d = nc.declare_dram_parameter("wq8", [128, 2 * KC * LM], F8,
                                      isOutput=False)
    mem_d = nc.declare_dram_parameter("mem", [128, MH * D], BF16, isOutput=False)
    # rowc: bq+bm row | mask row | C1*wst row
    rowc_d = nc.declare_dram_parameter("rowc", [1, 2 * D + LM], BF16,
                                       isOutput=False)
    out_d = nc.declare_dram_parameter("out", [LQ, D + 4], F32, isOutput=True)

    with tile.TileContext(nc) as tc:
        with (
            tc.tile_pool(name="const", bufs=1) as const,
            tc.tile_pool(name="io", bufs=1) as io,
            tc.tile_pool(name="lad", bufs=1) as lad,
            tc.tile_pool(name="ps_q0", bufs=1, space="PSUM") as ps_q0,
            tc.tile_pool(name="ps_q1", bufs=1, space="PSUM") as ps_q1,
            tc.tile_pool(name="ps_m", bufs=1, space="PSUM") as ps_m,
            tc.tile_pool(name="ps_s0", bufs=1, space="PSUM") as ps_s0,
            tc.tile_pool(name="ps_s1", bufs=1, space="PSUM") as ps_s1,
            tc.tile_pool(name="ps_o", bufs=1, space="PSUM") as ps_o,
            tc.tile_pool(name="ps_r", bufs=1, space="PSUM") as ps_r,
        ):
            V = nc.vector
            G = nc.gpsimd
            A = nc.scalar
            T = nc.tensor

            def cs(c, w=128):
                return slice(c * w, (c + 1) * w)

            # ---- DMA triggers: me-path first on every queue -------------
            # sin table preload leads the scalar queue (overlaps DMA wait)
            dummy = const.tile([128, 1], F32, tag="dummy")
            V.memset(dummy[:], 0.0)
            A.activation(dummy[:], dummy[:], AF.Sin)

            # three queues (each has its own DMA engine set), me-path first
            wmL = io.tile([128, KC * LM], F8, tag="wmL")
            A.dma_start(wmL[:], wmL_d[:])
            mT = io.tile([128, KC * LM], BF16, tag="mT")
            nc.sync.dma_start(mT[:], mT_d[:])
            wq8 = io.tile([128, 2 * KC * LM], F8, tag="wq8")
            nc.sync.dma_start(wq8[:], wq8_d[:])
            wmR = io.tile([128, KC * LM], F8, tag="wmR")
            G.dma_start(wmR[:], wmR_d[:])
            qT = io.tile([128, D], BF16, tag="qT")
            G.dma_start(qT[:], qT_d[:])
            rowc = const.tile([1, 2 * D + LM], BF16, tag="rowc")
            G.dma_start(rowc[:], rowc_d[:])
            mem_t = io.tile([128, MH * D], BF16, tag="mem_t")
            G.dma_start(mem_t[:], mem_d[:])

            wqL = wq8[:, 0:KC * LM]
            wqR = wq8[:, KC * LM:2 * KC * LM]
            bsum = rowc[:, 0:D]          # bq+bm row
            maskv = rowc[:, D:D + LM]    # MASK_NEG*(mask-1) row
            wrow = rowc[:, D + LM:2 * D + LM]   # C1*wst row

            # ---- on-chip consts (DVE idle during load) ------------------
            pihalf = const.tile([128, 1], F32, tag="pihalf")
            V.memset(pihalf[:], PIH)
            ones1 = const.tile([1, 128], BF16, tag="ones1")
            V.memset(ones1[:], 1.0)
            onesc = const.tile([128, 1], BF16, tag="onesc")
            V.memset(onesc[:], 1.0)
            o_sb = io.tile([128, D + 4], F32, tag="o_sb")
            V.memset(o_sb[:, D + 1:D + 4], 0.0)


            # ---- tiles ---------------------------------------------------
            MS, QS = [128, KC * LM], [128, D]
            MHS = [slice(0, 2 * LM), slice(2 * LM, 4 * LM)]
            QHS = [slice(0, 2 * LQ), slice(2 * LQ, 4 * LQ)]

            def mk(shape, tag):
                return lad.tile(shape, BF16, tag=tag, name=tag)

            s1m, c1m = mk(MS, "s1m"), mk(MS, "c1m")
            tm, dp1m, dm1m = mk(MS, "tm"), mk(MS, "dp1m"), mk(MS, "dm1m")
            s3m, c3m = mk(MS, "s3m"), mk(MS, "c3m")
            s1q, c1q = mk(QS, "s1q"), mk(QS, "c1q")
            s1qw, c1qw, uq = mk(QS, "s1qw"), mk(QS, "c1qw"), mk(QS, "uq")
            dp1q, dm1q = mk(QS, "dp1q"), mk(QS, "dm1q")
            s3qw, c3qw = mk(QS, "s3qw"), mk(QS, "c3qw")

            ps_me = ps_m.tile([128, KC * LM], F32, tag="ps_me")
            ps_qe = [ps_q0.tile([128, 2 * LQ], F32, tag="ps_qe0", name="qe0"),
                     ps_q1.tile([128, 2 * LQ], F32, tag="ps_qe1", name="qe1")]


            # ---- W512[p, c*128+i] = C1*wst[c*128+p]: rank-1s on idle PE -
            W512 = const.tile([128, D], BF16, tag="W512")
            for h in range(2):
                for ci in range(2):
                    T.matmul(ps_qe[h][:, cs(ci)], wrow[:, cs(2 * h + ci)],
                             ones1[:], start=(ci == 0), stop=(ci == 1))
                V.tensor_copy(W512[:, QHS[h]], ps_qe[h][:])

            # ---- me bank0 (d-chunks 0,1) + its sins/ladder --------------
            def me_bank(half):
                sl = MHS[half]
                w = wmL if half == 0 else wmR
                for k in range(KC):
                    for ci in range(2):
                        c = 2 * half + ci
                        T.matmul(ps_me[:, cs(c, LM)],
                                 w[:, k * LM + ci * 128:k * LM + ci * 128 + 128],
                                 mT[:, cs(k, LM)], start=(k == 0 and ci == 0),
                                 stop=(k == KC - 1 and ci == 1))
                with tc.high_priority(offset=400 - 10 * half):
                    A.activation(s1m[:, sl], ps_me[:, sl], AF.Sin, scale=W)
                    A.activation(c1m[:, sl], ps_me[:, sl], AF.Sin,
                                 bias=pihalf[:], scale=W)
                V.tensor_tensor(tm[:, sl], s1m[:, sl], s1m[:, sl], OP.mult)
                G.tensor_scalar(dp1m[:, sl], tm[:, sl], -4.0, 3.0,
                                OP.mult, OP.add)
                G.tensor_scalar(dm1m[:, sl], tm[:, sl], -4.0, 1.0,
                                OP.mult, OP.add)
                V.tensor_tensor(s3m[:, sl], dp1m[:, sl], s1m[:, sl], OP.mult)
                V.tensor_tensor(c3m[:, sl], dm1m[:, sl], c1m[:, sl], OP.mult)

            def qe_half(half):
                q = QHS[half]
                wq_h = wqL if half == 0 else wqR
                for k in range(KC):
                    for ci in range(2):
                        T.matmul(ps_qe[half][:, cs(ci)],
                                 wq_h[:, k * LM + ci * 128:k * LM + ci * 128 + 128],
                                 qT[:, cs(k)], start=(k == 0 and ci == 0),
                                 stop=False)
                for ci in range(2):  # bias rank-1s close the bank
                    T.matmul(ps_qe[half][:, cs(ci)],
                             bsum[:, cs(2 * half + ci)], ones1[:],
                             start=False, stop=(ci == 1))
                with tc.high_priority(offset=300 - 10 * half):
                    A.activation(s1q[:, q], ps_qe[half][:], AF.Sin, scale=W)
                    A.activation(c1q[:, q], ps_qe[half][:], AF.Sin,
                                 bias=pihalf[:], scale=W)
                V.tensor_tensor(s1qw[:, q], s1q[:, q], W512[:, q], OP.mult)
                V.tensor_tensor(uq[:, q], s1q[:, q], s1q[:, q], OP.mult)
                V.tensor_tensor(c1qw[:, q], c1q[:, q], W512[:, q], OP.mult)
                G.tensor_scalar(dp1q[:, q], uq[:, q], -4.0 * R31, 3.0 * R31,
                                OP.mult, OP.add)
                G.tensor_scalar(dm1q[:, q], uq[:, q], -4.0 * R31, 1.0 * R31,
                                OP.mult, OP.add)
                V.tensor_tensor(s3qw[:, q], dp1q[:, q], s1qw[:, q], OP.mult)
                V.tensor_tensor(c3qw[:, q], dm1q[:, q], c1qw[:, q], OP.mult)

            me_bank(0)
            me_bank(1)
            qe_half(0)
            qe_half(1)

            # exp table preload, pinned after every Sin via a dep-merge op
            tdep = const.tile([128, 1], BF16, tag="tdep")
            V.tensor_tensor(tdep[:], c1q[:, D - 1:D], c1m[:, KC * LM - 1:KC * LM],
                            OP.add)
            dummy2 = const.tile([128, 1], F32, tag="dummy2")
            A.activation(dummy2[:], tdep[:], AF.Exp)

            # ---- S^T score matmuls: all of m-half 0 first ---------------
            pairs = [(c1m, s1qw), (s1m, c1qw), (c3m, s3qw), (s3m, c3qw)]
            sps = [ps_s0.tile([128, 128], F32, tag="sps0", name="sps0"),
                   ps_s1.tile([128, 128], F32, tag="sps1", name="sps1")]
            expmT = [io.tile([128, 128], BF16, tag="expT0", name="expT0"),
                     io.tile([128, 128], BF16, tag="expT1", name="expT1")]
            o_ps = ps_o.tile([128, D], F32, tag="o_ps")
            r_ps = ps_r.tile([128, 1], F32, tag="r_ps")

            for h in (0, 1):
                first = True
                for mt, qt in pairs:
                    for c in range(KC):
                        T.matmul(sps[h][:],
                                 mt[:, c * LM + h * 128:c * LM + h * 128 + 128],
                                 qt[:, cs(c)], start=first, stop=False)
                        first = False
                T.matmul(sps[h][:], maskv[:, cs(h)], ones1[:],
                         start=False, stop=True)
                A.activation(expmT[h][:], sps[h][:], AF.Exp)
                T.matmul(o_ps[:], expmT[h][:], mem_t[:, h * D:(h + 1) * D],
                         start=(h == 0), stop=(h == 1))
                T.matmul(r_ps[:], expmT[h][:], onesc[:],
                         start=(h == 0), stop=(h == 1))

            # ---- store unnormalized + row-sum, two overlapped DMAs ------
            A.activation(o_sb[:, 0:D // 2], o_ps[:, 0:D // 2], AF.Copy)
            nc.sync.dma_start(out_d[:, 0:D // 2], o_sb[:, 0:D // 2])
            V.tensor_copy(o_sb[:, D // 2:D], o_ps[:, D // 2:D])
            V.tensor_copy(o_sb[:, D:D + 1], r_ps[:])
            nc.sync.dma_start(out_d[:, D // 2:D + 4], o_sb[:, D // 2:D + 4])

    nc.compile()
    return nc


_NC = None


def _get_nc() -> bass.Bass:
    global _NC
    if _NC is None:
        _NC = _build()
    return _NC


def _prep(x, dt=ml_dtypes.bfloat16):
    return np.ascontiguousarray(np.asarray(x, dtype=np.float32)).astype(dt)


def _make_in_maps(inputs):
    query = np.asarray(inputs["query"], np.float32)    # [B, LQ, D]
    memory = np.asarray(inputs["memory"], np.float32)  # [B, LM, D]
    Wq = np.asarray(inputs["Wq"], np.float32)
    bq = np.asarray(inputs["bq"], np.float32)
    Wm = np.asarray(inputs["Wm"], np.float32)
    bm = np.asarray(inputs["bm"], np.float32)
    wst = np.asarray(inputs["wst"], np.float32)
    mask = np.asarray(inputs["memory_mask"]).astype(np.float32)  # [B, LM]

    # d_out-major weight layouts (see kernel docstring)
    wm3 = Wm.reshape(KC, 128, D).transpose(1, 0, 2)   # [128, k, d_out]
    wq3 = Wq.reshape(KC, 128, D).transpose(1, 0, 2)
    wmL_h = _prep(wm3[:, :, 0:LM].reshape(128, KC * LM), ml_dtypes.float8_e4m3)
    wmR_h = _prep(wm3[:, :, LM:D].reshape(128, KC * LM), ml_dtypes.float8_e4m3)
    wqL = wq3[:, :, 0:LM].reshape(128, KC * LM)
    wqR = wq3[:, :, LM:D].reshape(128, KC * LM)
    bsum = (bq + bm).reshape(1, D)
    wrow = (C1 * wst).reshape(1, D)

    maps = []
    for b in range(B):
        mT = _prep(memory[b].T.reshape(KC, 128, LM).transpose(1, 0, 2)
                   .reshape(128, KC * LM))
        rowc = np.concatenate(
            [bsum, (MASK_NEG * (mask[b] - 1.0)).reshape(1, LM), wrow], axis=1)
        maps.append({
            "mT": mT,
            "wmL": wmL_h,
            "wmR": wmR_h,
            "qT": _prep(query[b].T.reshape(KC, 128, LQ).transpose(1, 0, 2)
                        .reshape(128, KC * LQ)),
            "wq8": _prep(np.concatenate([wqL, wqR], axis=1),
                         ml_dtypes.float8_e4m3),
            "mem": _prep(memory[b].reshape(MH, 128, D).transpose(1, 0, 2)
                         .reshape(128, MH * D)),
            "rowc": _prep(rowc),
        })
    return maps


def run_raw(inputs, **kwargs):
    """Run and return the full BassKernelResults (for profiling from test.py)."""
    nc = _get_nc()
    return run_bass_kernel_spmd(nc, _make_in_maps(inputs), list(range(B)), **kwargs)


def kernel(**inputs) -> np.ndarray:
    res = run_raw(inputs)
    outs = []
    for b in range(B):
        o = np.asarray(res.results[b]["out"], np.float32)
        outs.append(o[:, 0:D] / o[:, D:D + 1])
    return np.stack(outs).astype(np.float32)


if __name__ == "__main__":
    nc = _get_nc()
    print("built ok")


# revision 39
# speedup vs baseline: 1.2326x; 1.2326x over previous
"""Trainium2 Bass kernel for additive (Bahdanau-style) attention aggregation.

Reference per batch b:
    qe = query @ Wq + bq; me = memory @ Wm + bm
    S[q,m] = sum_d wst[d] * tanh(qe[q,d] + me[m,d])
    out = softmax(S, m) @ memory

Sharding: data-parallel over batch B=8, one element per NeuronCore.

Algorithm: tanh(x) ~= C1 sin(Wx) + C3 sin(3Wx) fitted with a Gaussian-
density weight on the data's x-range (|x|<=4.7); each sin(kW(a+b))
separates into sin/cos products, so the score matrix is 4 rank-512
matmul terms on the PE. sin3/cos3 come from a short Chebyshev ladder
(sin3 = (3-4sin^2)sin, cos3 = (1-4sin^2)cos) with products on DVE and
scalar-linear steps on GpSimd. Scores are computed TRANSPOSED ([m,q] in
two PSUM half-tiles) so exp(S^T) feeds the output matmul directly as
lhsT -- no PE transposes; the softmax row-sum falls out of an extra
ones-column matmul. The output leaves PSUM unnormalized together with
the row-sums; the host divides. Weights are laid out d_out-major and
k-split so each PSUM bank's encoder inputs arrive as early as possible,
and every consumer is emitted immediately after its producer group
(the Tile scheduler honors emission order per engine and tracks
dependencies against writers-emitted-so-far).
"""

import os
import numpy as np
import ml_dtypes

import concourse.bass as bass
import concourse.bacc as bacc
import concourse.tile as tile
from concourse import mybir
from concourse.bass_utils import run_bass_kernel_spmd

F32 = mybir.dt.float32
BF16 = mybir.dt.bfloat16
F8 = mybir.dt.float8e4
AF = mybir.ActivationFunctionType
OP = mybir.AluOpType

B = 8
LQ = 128
LM = 256
D = 512
KC = D // 128   # d-model chunks
MH = LM // 128  # memory partition chunks
PIH = float(np.pi / 2)

# tanh(x) ~= C1 sin(Wx) + C3 sin(3Wx), density-weighted fit on |x|<=4.7
W = 0.686790
C1, C3 = 1.056331, 0.115109
if os.environ.get("KERNEL_SIM_SAFE"):  # CoreSim asserts |sin arg| <= pi;
    W = 0.54926                        # HW degrades gracefully past pi
    C1, C3 = 1.114898, 0.19142
R31 = C3 / C1
MASK_NEG = 50.0


def _build() -> bass.Bass:
    nc = bacc.Bacc("TRN2", target_bir_lowering=False)

    # wm: d_out-major halves (separate completions); qbig: qT|wqL|wqR
    mT_d = nc.declare_dram_parameter("mT", [128, KC * LM], BF16, isOutput=False)
    wmL_d = nc.declare_dram_parameter("wmL", [128, KC * LM], F8, isOutput=False)
    wmR_d = nc.declare_dram_parameter("wmR", [128, KC * LM], F8, isOutput=False)
    qT_d = nc.declare_dram_parameter("qT", [128, D], BF16, isOutput=False)
    wq8_d = nc.declare_dram_parameter("wq8", [128, 2 * KC * LM], F8,
                                      isOutput=False)
    mem_d = nc.declare_dram_parameter("mem", [128, MH * D], BF16, isOutput=False)
    # rowc: bq+bm row | mask row | C1*wst row
    rowc_d = nc.declare_dram_parameter("rowc", [1, 2 * D + LM], BF16,
                                       isOutput=False)
    out_d = nc.declare_dram_parameter("out", [LQ, D + 4], F32, isOutput=True)

    with tile.TileContext(nc) as tc:
        with (
            tc.tile_pool(name="const", bufs=1) as const,
            tc.tile_pool(name="io", bufs=1) as io,
            tc.tile_pool(name="lad", bufs=1) as lad,
            tc.tile_pool(name="ps_q0", bufs=1, space="PSUM") as ps_q0,
            tc.tile_pool(name="ps_q1", bufs=1, space="PSUM") as ps_q1,
            tc.tile_pool(name="ps_m", bufs=1, space="PSUM") as ps_m,
            tc.tile_pool(name="ps_s0", bufs=1, space="PSUM") as ps_s0,
            tc.tile_pool(name="ps_s1", bufs=1, space="PSUM") as ps_s1,
            tc.tile_pool(name="ps_o", bufs=1, space="PSUM") as ps_o,
            tc.tile_pool(name="ps_r", bufs=1, space="PSUM") as ps_r,
        ):
            V = nc.vector
            G = nc.gpsimd
            A = nc.scalar
            T = nc.tensor

            def cs(c, w=128):
                return slice(c * w, (c + 1) * w)

            # ---- DMA triggers: me-path first on every queue -------------
            # sin table preload leads the scalar queue (overlaps DMA wait)
            dummy = const.tile([128, 1], F32, tag="dummy")
            V.memset(dummy[:], 0.0)
            A.activation(dummy[:], dummy[:], AF.Sin)

            # bulk data ONLY on the two hardware-DGE queues (sync/scalar);
            # the gpsimd queue generates descriptors in software (slow) and
            # carries just the tiny const row
            wmL = io.tile([128, KC * LM], F8, tag="wmL")
            A.dma_start(wmL[:], wmL_d[:])
            wmR = io.tile([128, KC * LM], F8, tag="wmR")
            A.dma_start(wmR[:], wmR_d[:])
            qT = io.tile([128, D], BF16, tag="qT")
            A.dma_start(qT[:], qT_d[:])
            mem_t = io.tile([128, MH * D], BF16, tag="mem_t")
            A.dma_start(mem_t[:], mem_d[:])
            mT = io.tile([128, KC * LM], BF16, tag="mT")
            nc.sync.dma_start(mT[:], mT_d[:])
            wq8 = io.tile([128, 2 * KC * LM], F8, tag="wq8")
            nc.sync.dma_start(wq8[:], wq8_d[:])
            rowc = const.tile([1, 2 * D + LM], BF16, tag="rowc")
            G.dma_start(rowc[:], rowc_d[:])

            wqL = wq8[:, 0:KC * LM]
            wqR = wq8[:, KC * LM:2 * KC * LM]
            bsum = rowc[:, 0:D]          # bq+bm row
            maskv = rowc[:, D:D + LM]    # MASK_NEG*(mask-1) row
            wrow = rowc[:, D + LM:2 * D + LM]   # C1*wst row

            # ---- on-chip consts (DVE idle during load) ------------------
            pihalf = const.tile([128, 1], F32, tag="pihalf")
            V.memset(pihalf[:], PIH)
            ones1 = const.tile([1, 128], BF16, tag="ones1")
            V.memset(ones1[:], 1.0)
            onesc = const.tile([128, 1], BF16, tag="onesc")
            V.memset(onesc[:], 1.0)
            o_sb = io.tile([128, D + 4], F32, tag="o_sb")
            V.memset(o_sb[:, D + 1:D + 4], 0.0)


            # ---- tiles ---------------------------------------------------
            MS, QS = [128, KC * LM], [128, D]
            MHS = [slice(0, 2 * LM), slice(2 * LM, 4 * LM)]
            QHS = [slice(0, 2 * LQ), slice(2 * LQ, 4 * LQ)]

            def mk(shape, tag):
                return lad.tile(shape, BF16, tag=tag, name=tag)

            s1m, c1m = mk(MS, "s1m"), mk(MS, "c1m")
            tm, dp1m, dm1m = mk(MS, "tm"), mk(MS, "dp1m"), mk(MS, "dm1m")
            s3m, c3m = mk(MS, "s3m"), mk(MS, "c3m")
            s1q, c1q = mk(QS, "s1q"), mk(QS, "c1q")
            s1qw, c1qw, uq = mk(QS, "s1qw"), mk(QS, "c1qw"), mk(QS, "uq")
            dp1q, dm1q = mk(QS, "dp1q"), mk(QS, "dm1q")
            s3qw, c3qw = mk(QS, "s3qw"), mk(QS, "c3qw")

            ps_me = ps_m.tile([128, KC * LM], F32, tag="ps_me")
            ps_qe = [ps_q0.tile([128, 2 * LQ], F32, tag="ps_qe0", name="qe0"),
                     ps_q1.tile([128, 2 * LQ], F32, tag="ps_qe1", name="qe1")]


            # ---- W512[p, c*128+i] = C1*wst[c*128+p]: rank-1s on idle PE -
            W512 = const.tile([128, D], BF16, tag="W512")
            for h in range(2):
                for ci in range(2):
                    T.matmul(ps_qe[h][:, cs(ci)], wrow[:, cs(2 * h + ci)],
                             ones1[:], start=(ci == 0), stop=(ci == 1))
                V.tensor_copy(W512[:, QHS[h]], ps_qe[h][:])

            # ---- me bank0 (d-chunks 0,1) + its sins/ladder --------------
            def me_bank(half):
                sl = MHS[half]
                w = wmL if half == 0 else wmR
                for k in range(KC):
                    for ci in range(2):
                        c = 2 * half + ci
                        T.matmul(ps_me[:, cs(c, LM)],
                                 w[:, k * LM + ci * 128:k * LM + ci * 128 + 128],
                                 mT[:, cs(k, LM)], start=(k == 0 and ci == 0),
                                 stop=(k == KC - 1 and ci == 1))
                with tc.high_priority(offset=400 - 10 * half):
                    A.activation(s1m[:, sl], ps_me[:, sl], AF.Sin, scale=W)
                    A.activation(c1m[:, sl], ps_me[:, sl], AF.Sin,
                                 bias=pihalf[:], scale=W)
                V.tensor_tensor(tm[:, sl], s1m[:, sl], s1m[:, sl], OP.mult)
                G.tensor_scalar(dp1m[:, sl], tm[:, sl], -4.0, 3.0,
                                OP.mult, OP.add)
                G.tensor_scalar(dm1m[:, sl], tm[:, sl], -4.0, 1.0,
                                OP.mult, OP.add)
                V.tensor_tensor(s3m[:, sl], dp1m[:, sl], s1m[:, sl], OP.mult)
                V.tensor_tensor(c3m[:, sl], dm1m[:, sl], c1m[:, sl], OP.mult)

            def qe_half(half):
                q = QHS[half]
                wq_h = wqL if half == 0 else wqR
                for k in range(KC):
                    for ci in range(2):
                        T.matmul(ps_qe[half][:, cs(ci)],
                                 wq_h[:, k * LM + ci * 128:k * LM + ci * 128 + 128],
                                 qT[:, cs(k)], start=(k == 0 and ci == 0),
                                 stop=False)
                for ci in range(2):  # bias rank-1s close the bank
                    T.matmul(ps_qe[half][:, cs(ci)],
                             bsum[:, cs(2 * half + ci)], ones1[:],
                             start=False, stop=(ci == 1))
                with tc.high_priority(offset=300 - 10 * half):
                    A.activation(s1q[:, q], ps_qe[half][:], AF.Sin, scale=W)
                    A.activation(c1q[:, q], ps_qe[half][:], AF.Sin,
                                 bias=pihalf[:], scale=W)
                V.tensor_tensor(s1qw[:, q], s1q[:, q], W512[:, q], OP.mult)
                V.tensor_tensor(uq[:, q], s1q[:, q], s1q[:, q], OP.mult)
                V.tensor_tensor(c1qw[:, q], c1q[:, q], W512[:, q], OP.mult)
                G.tensor_scalar(dp1q[:, q], uq[:, q], -4.0 * R31, 3.0 * R31,
                                OP.mult, OP.add)
                G.tensor_scalar(dm1q[:, q], uq[:, q], -4.0 * R31, 1.0 * R31,
                                OP.mult, OP.add)
                V.tensor_tensor(s3qw[:, q], dp1q[:, q], s1qw[:, q], OP.mult)
                V.tensor_tensor(c3qw[:, q], dm1q[:, q], c1qw[:, q], OP.mult)

            me_bank(0)
            me_bank(1)
            qe_half(0)
            qe_half(1)

            # exp table preload, pinned after every Sin via a dep-merge op
            tdep = const.tile([128, 1], BF16, tag="tdep")
            V.tensor_tensor(tdep[:], c1q[:, D - 1:D], c1m[:, KC * LM - 1:KC * LM],
                            OP.add)
            dummy2 = const.tile([128, 1], F32, tag="dummy2")
            A.activation(dummy2[:], tdep[:], AF.Exp)

            # ---- S^T score matmuls: all of m-half 0 first ---------------
            pairs = [(c1m, s1qw), (s1m, c1qw), (c3m, s3qw), (s3m, c3qw)]
            sps = [ps_s0.tile([128, 128], F32, tag="sps0", name="sps0"),
                   ps_s1.tile([128, 128], F32, tag="sps1", name="sps1")]
            expmT = [io.tile([128, 128], BF16, tag="expT0", name="expT0"),
                     io.tile([128, 128], BF16, tag="expT1", name="expT1")]
            o_ps = ps_o.tile([128, D], F32, tag="o_ps")
            r_ps = ps_r.tile([128, 1], F32, tag="r_ps")

            for h in (0, 1):
                first = True
                for mt, qt in pairs:
                    for c in range(KC):
                        T.matmul(sps[h][:],
                                 mt[:, c * LM + h * 128:c * LM + h * 128 + 128],
                                 qt[:, cs(c)], start=first, stop=False)
                        first = False
                T.matmul(sps[h][:], maskv[:, cs(h)], ones1[:],
                         start=False, stop=True)
                A.activation(expmT[h][:], sps[h][:], AF.Exp)
                T.matmul(o_ps[:], expmT[h][:], mem_t[:, h * D:(h + 1) * D],
                         start=(h == 0), stop=(h == 1))
                T.matmul(r_ps[:], expmT[h][:], onesc[:],
                         start=(h == 0), stop=(h == 1))

            # ---- store unnormalized + row-sum, two overlapped DMAs ------
            A.activation(o_sb[:, 0:D // 2], o_ps[:, 0:D // 2], AF.Copy)
            nc.sync.dma_start(out_d[:, 0:D // 2], o_sb[:, 0:D // 2])
            V.tensor_copy(o_sb[:, D // 2:D], o_ps[:, D // 2:D])
            V.tensor_copy(o_sb[:, D:D + 1], r_ps[:])
            nc.sync.dma_start(out_d[:, D // 2:D + 4], o_sb[:, D // 2:D + 4])

    nc.compile()
    return nc


_NC = None


def _get_nc() -> bass.Bass:
    global _NC
    if _NC is None:
        _NC = _build()
    return _NC


def _prep(x, dt=ml_dtypes.bfloat16):
    return np.ascontiguousarray(np.asarray(x, dtype=np.float32)).astype(dt)


def _make_in_maps(inputs):
    query = np.asarray(inputs["query"], np.float32)    # [B, LQ, D]
    memory = np.asarray(inputs["memory"], np.float32)  # [B, LM, D]
    Wq = np.asarray(inputs["Wq"], np.float32)
    bq = np.asarray(inputs["bq"], np.float32)
    Wm = np.asarray(inputs["Wm"], np.float32)
    bm = np.asarray(inputs["bm"], np.float32)
    wst = np.asarray(inputs["wst"], np.float32)
    mask = np.asarray(inputs["memory_mask"]).astype(np.float32)  # [B, LM]

    # d_out-major weight layouts (see kernel docstring)
    wm3 = Wm.reshape(KC, 128, D).transpose(1, 0, 2)   # [128, k, d_out]
    wq3 = Wq.reshape(KC, 128, D).transpose(1, 0, 2)
    wmL_h = _prep(wm3[:, :, 0:LM].reshape(128, KC * LM), ml_dtypes.float8_e4m3)
    wmR_h = _prep(wm3[:, :, LM:D].reshape(128, KC * LM), ml_dtypes.float8_e4m3)
    wqL = wq3[:, :, 0:LM].reshape(128, KC * LM)
    wqR = wq3[:, :, LM:D].reshape(128, KC * LM)
    bsum = (bq + bm).reshape(1, D)
    wrow = (C1 * wst).reshape(1, D)

    maps = []
    for b in range(B):
        mT = _prep(memory[b].T.reshape(KC, 128, LM).transpose(1, 0, 2)
                   .reshape(128, KC * LM))
        rowc = np.concatenate(
            [bsum, (MASK_NEG * (mask[b] - 1.0)).reshape(1, LM), wrow], axis=1)
        maps.append({
            "mT": mT,
            "wmL": wmL_h,
            "wmR": wmR_h,
            "qT": _prep(query[b].T.reshape(KC, 128, LQ).transpose(1, 0, 2)
                        .reshape(128, KC * LQ)),
            "wq8": _prep(np.concatenate([wqL, wqR], axis=1),
                         ml_dtypes.float8_e4m3),
            "mem": _prep(memory[b].reshape(MH, 128, D).transpose(1, 0, 2)
                         .reshape(128, MH * D)),
            "rowc": _prep(rowc),
        })
    return maps


def run_raw(inputs, **kwargs):
    """Run and return the full BassKernelResults (for profiling from test.py)."""
    nc = _get_nc()
    return run_bass_kernel_spmd(nc, _make_in_maps(inputs), list(range(B)), **kwargs)


def kernel(**inputs) -> np.ndarray:
    res = run_raw(inputs)
    outs = []
    for b in range(B):
        o = np.asarray(res.results[b]["out"], np.float32)
        outs.append(o[:, 0:D] / o[:, D:D + 1])
    return np.stack(outs).astype(np.float32)


if __name__ == "__main__":
    nc = _get_nc()
    print("built ok")


# revision 40
# speedup vs baseline: 1.2359x; 1.0026x over previous
"""Trainium2 Bass kernel for additive (Bahdanau-style) attention aggregation.

Reference per batch b:
    qe = query @ Wq + bq; me = memory @ Wm + bm
    S[q,m] = sum_d wst[d] * tanh(qe[q,d] + me[m,d])
    out = softmax(S, m) @ memory

Sharding: data-parallel over batch B=8, one element per NeuronCore.

Algorithm: tanh(x) ~= C1 sin(Wx) + C3 sin(3Wx) fitted with a Gaussian-
density weight on the data's x-range (|x|<=4.7); each sin(kW(a+b))
separates into sin/cos products, so the score matrix is 4 rank-512
matmul terms on the PE. sin3/cos3 come from a short Chebyshev ladder
(sin3 = (3-4sin^2)sin, cos3 = (1-4sin^2)cos) with products on DVE and
scalar-linear steps on GpSimd. Scores are computed TRANSPOSED ([m,q] in
two PSUM half-tiles) so exp(S^T) feeds the output matmul directly as
lhsT -- no PE transposes; the softmax row-sum falls out of an extra
ones-column matmul. The output leaves PSUM unnormalized together with
the row-sums; the host divides. Weights are laid out d_out-major and
k-split so each PSUM bank's encoder inputs arrive as early as possible,
and every consumer is emitted immediately after its producer group
(the Tile scheduler honors emission order per engine and tracks
dependencies against writers-emitted-so-far).
"""

import os
import numpy as np
import ml_dtypes

import concourse.bass as bass
import concourse.bacc as bacc
import concourse.tile as tile
from concourse import mybir
from concourse.bass_utils import run_bass_kernel_spmd

F32 = mybir.dt.float32
BF16 = mybir.dt.bfloat16
F8 = mybir.dt.float8e4
AF = mybir.ActivationFunctionType
OP = mybir.AluOpType

B = 8
LQ = 128
LM = 256
D = 512
KC = D // 128   # d-model chunks
MH = LM // 128  # memory partition chunks
PIH = float(np.pi / 2)

# tanh(x) ~= C1 sin(Wx) + C3 sin(3Wx), density-weighted fit on |x|<=4.7
W = 0.686790
C1, C3 = 1.056331, 0.115109
if os.environ.get("KERNEL_SIM_SAFE"):  # CoreSim asserts |sin arg| <= pi;
    W = 0.54926                        # HW degrades gracefully past pi
    C1, C3 = 1.114898, 0.19142
R31 = C3 / C1
MASK_NEG = 50.0


def _build() -> bass.Bass:
    nc = bacc.Bacc("TRN2", target_bir_lowering=False)

    # wm: d_out-major halves (separate completions); qbig: qT|wqL|wqR
    mT_d = nc.declare_dram_parameter("mT", [128, KC * LM], BF16, isOutput=False)
    wmL_d = nc.declare_dram_parameter("wmL", [128, KC * LM], F8, isOutput=False)
    wmR_d = nc.declare_dram_parameter("wmR", [128, KC * LM], F8, isOutput=False)
    qT_d = nc.declare_dram_parameter("qT", [128, D], BF16, isOutput=False)
    wq8_d = nc.declare_dram_parameter("wq8", [128, 2 * KC * LM], F8,
                                      isOutput=False)
    mem_d = nc.declare_dram_parameter("mem", [128, MH * D], BF16, isOutput=False)
    # rowc: bq+bm row | mask row | C1*wst row
    rowc_d = nc.declare_dram_parameter("rowc", [1, 2 * D + LM], BF16,
                                       isOutput=False)
    out_d = nc.declare_dram_parameter("out", [LQ, D + 4], F32, isOutput=True)

    with tile.TileContext(nc) as tc:
        with (
            tc.tile_pool(name="const", bufs=1) as const,
            tc.tile_pool(name="io", bufs=1) as io,
            tc.tile_pool(name="lad", bufs=1) as lad,
            tc.tile_pool(name="ps_q0", bufs=1, space="PSUM") as ps_q0,
            tc.tile_pool(name="ps_q1", bufs=1, space="PSUM") as ps_q1,
            tc.tile_pool(name="ps_m", bufs=1, space="PSUM") as ps_m,
            tc.tile_pool(name="ps_s0", bufs=1, space="PSUM") as ps_s0,
            tc.tile_pool(name="ps_s1", bufs=1, space="PSUM") as ps_s1,
            tc.tile_pool(name="ps_o", bufs=1, space="PSUM") as ps_o,
            tc.tile_pool(name="ps_r", bufs=1, space="PSUM") as ps_r,
        ):
            V = nc.vector
            G = nc.gpsimd
            A = nc.scalar
            T = nc.tensor

            def cs(c, w=128):
                return slice(c * w, (c + 1) * w)

            # ---- DMA triggers: me-path first on every queue -------------
            # sin table preload leads the scalar queue (overlaps DMA wait)
            dummy = const.tile([128, 1], F32, tag="dummy")
            V.memset(dummy[:], 0.0)
            A.activation(dummy[:], dummy[:], AF.Sin)

            # bulk data ONLY on the two hardware-DGE queues (sync/scalar);
            # the gpsimd queue generates descriptors in software (slow) and
            # carries just the tiny const row
            wmL = io.tile([128, KC * LM], F8, tag="wmL")
            A.dma_start(wmL[:], wmL_d[:])
            wmR = io.tile([128, KC * LM], F8, tag="wmR")
            A.dma_start(wmR[:], wmR_d[:])
            qT = io.tile([128, D], BF16, tag="qT")
            A.dma_start(qT[:], qT_d[:])
            mem_t = io.tile([128, MH * D], BF16, tag="mem_t")
            A.dma_start(mem_t[:], mem_d[:])
            mT = io.tile([128, KC * LM], BF16, tag="mT")
            nc.sync.dma_start(mT[:], mT_d[:])
            wq8 = io.tile([128, 2 * KC * LM], F8, tag="wq8")
            nc.sync.dma_start(wq8[:], wq8_d[:])
            rowc = const.tile([1, 2 * D + LM], BF16, tag="rowc")
            G.dma_start(rowc[:], rowc_d[:])

            wqL = wq8[:, 0:KC * LM]
            wqR = wq8[:, KC * LM:2 * KC * LM]
            bsum = rowc[:, 0:D]          # bq+bm row
            maskv = rowc[:, D:D + LM]    # MASK_NEG*(mask-1) row
            wrow = rowc[:, D + LM:2 * D + LM]   # C1*wst row

            # ---- on-chip consts (DVE idle during load) ------------------
            pihalf = const.tile([128, 1], F32, tag="pihalf")
            V.memset(pihalf[:], PIH)
            ones1 = const.tile([1, 128], BF16, tag="ones1")
            V.memset(ones1[:], 1.0)
            onesc = const.tile([128, 1], BF16, tag="onesc")
            V.memset(onesc[:], 1.0)
            o_sb = io.tile([128, D + 4], F32, tag="o_sb")
            V.memset(o_sb[:, D + 1:D + 4], 0.0)


            # ---- tiles ---------------------------------------------------
            MS, QS = [128, KC * LM], [128, D]
            MHS = [slice(0, 2 * LM), slice(2 * LM, 4 * LM)]
            QHS = [slice(0, 2 * LQ), slice(2 * LQ, 4 * LQ)]

            def mk(shape, tag):
                return lad.tile(shape, BF16, tag=tag, name=tag)

            s1m, c1m = mk(MS, "s1m"), mk(MS, "c1m")
            tm, dp1m, dm1m = mk(MS, "tm"), mk(MS, "dp1m"), mk(MS, "dm1m")
            s3m, c3m = mk(MS, "s3m"), mk(MS, "c3m")
            s1q, c1q = mk(QS, "s1q"), mk(QS, "c1q")
            s1qw, c1qw, uq = mk(QS, "s1qw"), mk(QS, "c1qw"), mk(QS, "uq")
            dp1q, dm1q = mk(QS, "dp1q"), mk(QS, "dm1q")
            s3qw, c3qw = mk(QS, "s3qw"), mk(QS, "c3qw")

            ps_me = ps_m.tile([128, KC * LM], F32, tag="ps_me")
            ps_qe = [ps_q0.tile([128, 2 * LQ], F32, tag="ps_qe0", name="qe0"),
                     ps_q1.tile([128, 2 * LQ], F32, tag="ps_qe1", name="qe1")]


            # ---- W512[p, c*128+i] = C1*wst[c*128+p]: rank-1s on idle PE -
            W512 = const.tile([128, D], BF16, tag="W512")
            for h in range(2):
                for ci in range(2):
                    T.matmul(ps_qe[h][:, cs(ci)], wrow[:, cs(2 * h + ci)],
                             ones1[:], start=(ci == 0), stop=(ci == 1))
                V.tensor_copy(W512[:, QHS[h]], ps_qe[h][:])

            # ---- me bank0 (d-chunks 0,1) + its sins/ladder --------------
            def me_bank(half):
                sl = MHS[half]
                w = wmL if half == 0 else wmR
                for k in range(KC):
                    for ci in range(2):
                        c = 2 * half + ci
                        T.matmul(ps_me[:, cs(c, LM)],
                                 w[:, k * LM + ci * 128:k * LM + ci * 128 + 128],
                                 mT[:, cs(k, LM)], start=(k == 0 and ci == 0),
                                 stop=(k == KC - 1 and ci == 1))
                with tc.high_priority(offset=400 - 10 * half):
                    A.activation(s1m[:, sl], ps_me[:, sl], AF.Sin, scale=W)
                    A.activation(c1m[:, sl], ps_me[:, sl], AF.Sin,
                                 bias=pihalf[:], scale=W)
                V.tensor_tensor(tm[:, sl], s1m[:, sl], s1m[:, sl], OP.mult)
                G.tensor_scalar(dp1m[:, sl], tm[:, sl], -4.0, 3.0,
                                OP.mult, OP.add)
                G.tensor_scalar(dm1m[:, sl], tm[:, sl], -4.0, 1.0,
                                OP.mult, OP.add)
                V.tensor_tensor(s3m[:, sl], dp1m[:, sl], s1m[:, sl], OP.mult)
                V.tensor_tensor(c3m[:, sl], dm1m[:, sl], c1m[:, sl], OP.mult)

            def qe_half(half):
                q = QHS[half]
                wq_h = wqL if half == 0 else wqR
                for k in range(KC):
                    for ci in range(2):
                        T.matmul(ps_qe[half][:, cs(ci)],
                                 wq_h[:, k * LM + ci * 128:k * LM + ci * 128 + 128],
                                 qT[:, cs(k)], start=(k == 0 and ci == 0),
                                 stop=False)
                for ci in range(2):  # bias rank-1s close the bank
                    T.matmul(ps_qe[half][:, cs(ci)],
                             bsum[:, cs(2 * half + ci)], ones1[:],
                             start=False, stop=(ci == 1))
                with tc.high_priority(offset=300 - 10 * half):
                    A.activation(s1q[:, q], ps_qe[half][:], AF.Sin, scale=W)
                    A.activation(c1q[:, q], ps_qe[half][:], AF.Sin,
                                 bias=pihalf[:], scale=W)
                V.tensor_tensor(uq[:, q], s1q[:, q], s1q[:, q], OP.mult)
                V.tensor_tensor(s1qw[:, q], s1q[:, q], W512[:, q], OP.mult)
                V.tensor_tensor(c1qw[:, q], c1q[:, q], W512[:, q], OP.mult)
                G.tensor_scalar(dp1q[:, q], uq[:, q], -4.0 * R31, 3.0 * R31,
                                OP.mult, OP.add)
                G.tensor_scalar(dm1q[:, q], uq[:, q], -4.0 * R31, 1.0 * R31,
                                OP.mult, OP.add)
                V.tensor_tensor(s3qw[:, q], dp1q[:, q], s1qw[:, q], OP.mult)
                V.tensor_tensor(c3qw[:, q], dm1q[:, q], c1qw[:, q], OP.mult)

            me_bank(0)
            me_bank(1)
            qe_half(0)
            qe_half(1)

            # exp table preload, pinned after every Sin via a dep-merge op
            tdep = const.tile([128, 1], BF16, tag="tdep")
            V.tensor_tensor(tdep[:], c1q[:, D - 1:D], c1m[:, KC * LM - 1:KC * LM],
                            OP.add)
            dummy2 = const.tile([128, 1], F32, tag="dummy2")
            A.activation(dummy2[:], tdep[:], AF.Exp)

            # ---- S^T score matmuls: all of m-half 0 first ---------------
            pairs = [(c1m, s1qw), (s1m, c1qw), (c3m, s3qw), (s3m, c3qw)]
            sps = [ps_s0.tile([128, 128], F32, tag="sps0", name="sps0"),
                   ps_s1.tile([128, 128], F32, tag="sps1", name="sps1")]
            expmT = [io.tile([128, 128], BF16, tag="expT0", name="expT0"),
                     io.tile([128, 128], BF16, tag="expT1", name="expT1")]
            o_ps = ps_o.tile([128, D], F32, tag="o_ps")
            r_ps = ps_r.tile([128, 1], F32, tag="r_ps")

            for h in (0, 1):
                T.matmul(sps[h][:], maskv[:, cs(h)], ones1[:],
                         start=True, stop=False)
                for mi, (mt, qt) in enumerate(pairs):
                    for c in range(KC):
                        T.matmul(sps[h][:],
                                 mt[:, c * LM + h * 128:c * LM + h * 128 + 128],
                                 qt[:, cs(c)], start=False,
                                 stop=(mi == len(pairs) - 1 and c == KC - 1))
                A.activation(expmT[h][:], sps[h][:], AF.Exp)
                T.matmul(o_ps[:], expmT[h][:], mem_t[:, h * D:(h + 1) * D],
                         start=(h == 0), stop=(h == 1))
                T.matmul(r_ps[:], expmT[h][:], onesc[:],
                         start=(h == 0), stop=(h == 1))

            # ---- store unnormalized + row-sum, two overlapped DMAs ------
            A.activation(o_sb[:, 0:D // 2], o_ps[:, 0:D // 2], AF.Copy)
            nc.sync.dma_start(out_d[:, 0:D // 2], o_sb[:, 0:D // 2])
            V.tensor_copy(o_sb[:, D // 2:D], o_ps[:, D // 2:D])
            V.tensor_copy(o_sb[:, D:D + 1], r_ps[:])
            nc.sync.dma_start(out_d[:, D // 2:D + 4], o_sb[:, D // 2:D + 4])

    nc.compile()
    return nc


_NC = None


def _get_nc() -> bass.Bass:
    global _NC
    if _NC is None:
        _NC = _build()
    return _NC


def _prep(x, dt=ml_dtypes.bfloat16):
    return np.ascontiguousarray(np.asarray(x, dtype=np.float32)).astype(dt)


def _make_in_maps(inputs):
    query = np.asarray(inputs["query"], np.float32)    # [B, LQ, D]
    memory = np.asarray(inputs["memory"], np.float32)  # [B, LM, D]
    Wq = np.asarray(inputs["Wq"], np.float32)
    bq = np.asarray(inputs["bq"], np.float32)
    Wm = np.asarray(inputs["Wm"], np.float32)
    bm = np.asarray(inputs["bm"], np.float32)
    wst = np.asarray(inputs["wst"], np.float32)
    mask = np.asarray(inputs["memory_mask"]).astype(np.float32)  # [B, LM]

    # d_out-major weight layouts (see kernel docstring)
    wm3 = Wm.reshape(KC, 128, D).transpose(1, 0, 2)   # [128, k, d_out]
    wq3 = Wq.reshape(KC, 128, D).transpose(1, 0, 2)
    wmL_h = _prep(wm3[:, :, 0:LM].reshape(128, KC * LM), ml_dtypes.float8_e4m3)
    wmR_h = _prep(wm3[:, :, LM:D].reshape(128, KC * LM), ml_dtypes.float8_e4m3)
    wqL = wq3[:, :, 0:LM].reshape(128, KC * LM)
    wqR = wq3[:, :, LM:D].reshape(128, KC * LM)
    bsum = (bq + bm).reshape(1, D)
    wrow = (C1 * wst).reshape(1, D)

    maps = []
    for b in range(B):
        mT = _prep(memory[b].T.reshape(KC, 128, LM).transpose(1, 0, 2)
                   .reshape(128, KC * LM))
        rowc = np.concatenate(
            [bsum, (MASK_NEG * (mask[b] - 1.0)).reshape(1, LM), wrow], axis=1)
        maps.append({
            "mT": mT,
            "wmL": wmL_h,
            "wmR": wmR_h,
            "qT": _prep(query[b].T.reshape(KC, 128, LQ).transpose(1, 0, 2)
                        .reshape(128, KC * LQ)),
            "wq8": _prep(np.concatenate([wqL, wqR], axis=1),
                         ml_dtypes.float8_e4m3),
            "mem": _prep(memory[b].reshape(MH, 128, D).transpose(1, 0, 2)
                         .reshape(128, MH * D)),
            "rowc": _prep(rowc),
        })
    return maps


def run_raw(inputs, **kwargs):
    """Run and return the full BassKernelResults (for profiling from test.py)."""
    nc = _get_nc()
    return run_bass_kernel_spmd(nc, _make_in_maps(inputs), list(range(B)), **kwargs)


def kernel(**inputs) -> np.ndarray:
    res = run_raw(inputs)
    outs = []
    for b in range(B):
        o = np.asarray(res.results[b]["out"], np.float32)
        outs.append(o[:, 0:D] / o[:, D:D + 1])
    return np.stack(outs).astype(np.float32)


if __name__ == "__main__":
    nc = _get_nc()
    print("built ok")
